# revision 3
# baseline (speedup 1.0000x reference)
"""Capsule routing kernel — nn_Capsule_28097676051143 (Trainium2 / Bass / Tile).

kernel(u_vecs [64,512,256] f32, W [1,256,2048] f32) -> [64, 32, 64] f32.

Data-parallel: batch 64 is split 8-per-core across 8 NeuronCores; W is
replicated. The routing is algebraically refactored so the 268 MB
u_hat = einsum('bie,end->bnid') tensor is never materialized:

    a[n,e]  = sum_i c[n,i] u[i,e]            (c @ u)
    o[n,d]  = a[n,:] @ W[:, n-block]         (diagonal of small matmul)
    g[n,e]  = W[:, n-block] @ o_norm[n,:]    (block-diagonal matmul)
    b[i,n]  = u[i,:] @ g[n,:]                (u @ g^T; softmax over n)

which is exact up to fp reassociation and cuts FLOPs ~4x vs
materializing u_hat. All heavy contractions run on the tensor engine in
bf16 with fp32 PSUM accumulation; softmax/normalization run on ACT/DVE;
iteration 0 exploits the uniform softmax (b=0) via a column-sum shortcut.

On-chip layouts (per core, SBUF; p = 128 partitions):
    u_sb   [ip, (ic, b, e)]   lhsT for a-phase
    uT_sb  [ep, (ec, b, i)]   lhsT for b-phase
    w_sb   [ep, (ec, nd)]     lhsT for o-phase
    wT_sb  [ndp, (ndc, e)]    lhsT for g-phase
    o_flat [ndp, (ndc, b)]    o in "flat capsule" layout, nd = n*64+d
"""

import functools
import numpy as np
import ml_dtypes

B, I, E, N, D = 64, 512, 256, 32, 64
NCORES, BPC = 8, 8
ND = N * D  # 2048
BF = ml_dtypes.bfloat16


def _build_module():
    import concourse.bass as bass
    import concourse.bacc as bacc
    import concourse.mybir as mybir
    import concourse.tile as tile
    from contextlib import ExitStack

    F32 = mybir.dt.float32
    BF16 = mybir.dt.bfloat16
    AX = mybir.AxisListType
    AF = mybir.ActivationFunctionType

    nc = bacc.Bacc("TRN2", target_bir_lowering=False, debug=False,
                   num_devices=NCORES)

    u_in = nc.dram_tensor("u_in", [128, 4 * BPC * E], BF16, kind="ExternalInput")
    uT_in = nc.dram_tensor("uT_in", [128, 2 * BPC * I], BF16, kind="ExternalInput")
    w_in = nc.dram_tensor("w_in", [128, 2 * ND], BF16, kind="ExternalInput")
    wT_in = nc.dram_tensor("wT_in", [128, 16 * E], BF16, kind="ExternalInput")
    out_d = nc.dram_tensor("out_d", [BPC, N, D], F32, kind="ExternalOutput")

    # mask[p, ndc*32 + n] = 1 iff n == 2*ndc + (p//64): sums squares of the
    # 64 d-partitions of capsule n into psum row n (s2 = ||o_n||^2).
    mask_np = np.zeros((128, 512), np.float32)
    for ndc in range(16):
        for p in range(128):
            mask_np[p, ndc * 32 + 2 * ndc + (p // 64)] = 1.0
    mask_dram = nc.inline_tensor(mask_np, name="mask_c")
    ident_dram = nc.inline_tensor(np.eye(128, dtype=np.float32), name="ident_c")

    with tile.TileContext(nc) as tc, ExitStack() as ctx:
        cp = ctx.enter_context(tc.tile_pool(name="const", bufs=1))
        wk = ctx.enter_context(tc.tile_pool(name="work", bufs=2))
        pp = ctx.enter_context(tc.tile_pool(name="psum", bufs=8, space="PSUM"))

        u_sb = cp.tile([128, 4 * BPC * E], BF16)
        nc.sync.dma_start(u_sb[:], u_in.ap())
        uT_sb = cp.tile([128, 2 * BPC * I], BF16)
        nc.sync.dma_start(uT_sb[:], uT_in.ap())
        w_sb = cp.tile([128, 2 * ND], BF16)
        nc.sync.dma_start(w_sb[:], w_in.ap())
        wT_sb = cp.tile([128, 16 * E], BF16)
        nc.sync.dma_start(wT_sb[:], wT_in.ap())
        mask_sb = cp.tile([128, 512], F32)
        nc.sync.dma_start(mask_sb[:], mask_dram.ap())
        ident_sb = cp.tile([128, 128], F32)
        nc.sync.dma_start(ident_sb[:], ident_dram.ap())

        # iteration-0 shortcut: c is uniform 1/N, so a[n,:] = usum/N for all n
        usum_f = cp.tile([128, 2 * BPC], F32)
        nc.vector.reduce_sum(usum_f[:],
                             uT_sb[:].rearrange("p (g i) -> p g i", i=I),
                             axis=AX.X)
        usum_sb = cp.tile([128, 2 * BPC], BF16)
        nc.scalar.mul(usum_sb[:], usum_f[:], 1.0 / N)

        bcast_shape = w_sb[:, 0:256].rearrange("p (b n) -> p b n", n=N)

        def rhs_iter0(ec):
            r = usum_sb[:, ec * BPC:(ec + 1) * BPC].rearrange(
                "p (b o) -> p b o", o=1)
            _, rb = bass.broadcast_tensor_aps(bcast_shape, r)
            return rb

        def emit_o_phase(rhs_for_ec):
            """o_flat[p, (ndc, b)] f32 <- diag blocks of aT.T @ W."""
            o_flat = wk.tile([128, 16 * BPC], F32, tag="oflat")
            for ndc in range(16):
                po = pp.tile([128, BPC * N], F32, tag="ps")
                for ec in range(2):
                    nc.tensor.matmul(
                        po[:],
                        w_sb[:, ec * ND + ndc * 128: ec * ND + (ndc + 1) * 128],
                        rhs_for_ec(ec), start=(ec == 0), stop=(ec == 1))
                v = po[:].rearrange("p (b n) -> p b n", n=N)
                nc.vector.tensor_copy(o_flat[0:64, ndc * 8:(ndc + 1) * 8],
                                      v[0:64, :, 2 * ndc])
                nc.scalar.copy(o_flat[64:128, ndc * 8:(ndc + 1) * 8],
                               v[64:128, :, 2 * ndc + 1])
            return o_flat

        def emit_s2(o_flat):
            """psum [32, 8]: s2[n, b] = sum_d o[n,d,b]^2."""
            sq = wk.tile([128, 16 * BPC], F32, tag="sq")
            nc.vector.tensor_mul(sq[:], o_flat[:], o_flat[:])
            ps2 = pp.tile([32, 8], F32, tag="ps")
            for ndc in range(16):
                nc.tensor.matmul(ps2[:],
                                 mask_sb[:, ndc * 32:(ndc + 1) * 32],
                                 sq[:, ndc * 8:(ndc + 1) * 8],
                                 start=(ndc == 0), stop=(ndc == 15))
            return ps2

        def emit_norm_g(o_flat):
            """gt[p, (eh, b, n)] bf16 = normalized g = W[:,n-blk] @ o_n / ||o_n||."""
            ps2 = emit_s2(o_flat)
            inv = wk.tile([32, 8], F32, tag="inv")
            nc.vector.reciprocal(inv[:], ps2[:])
            rs = wk.tile([32, 8], F32, tag="rs")      # 1/sqrt(s2)
            nc.scalar.sqrt(rs[:], inv[:])
            rsf = wk.tile([1, 256], F32, tag="rsf")   # flat (n*8 + b)
            nc.sync.dma_start(rsf[:], rs[:])
            rbe = wk.tile([128, 256], F32, tag="rbe")
            nc.gpsimd.partition_broadcast(rbe[:], rsf[:])

            # Z[p, (ndc, b, m')] bf16: block-diagonalized o for the g matmuls
            Z = wk.tile([128, 16 * BPC * 2], BF16, tag="Z")
            nc.vector.memset(Z[:], 0.0)
            zv = Z[:].rearrange("p (c two) -> p c two", two=2)
            nc.vector.tensor_copy(zv[0:64, :, 0], o_flat[0:64, :])
            nc.vector.tensor_copy(zv[64:128, :, 1], o_flat[64:128, :])

            # g: pgt[eh][p=e', (ndc, b, m')]
            pgt = [pp.tile([128, 256], F32, tag="ps", name=f"pgt{_eh}")
                   for _eh in range(2)]
            for ndc in range(16):
                for eh in range(2):
                    nc.tensor.matmul(
                        pgt[eh][:, ndc * 16:(ndc + 1) * 16],
                        wT_sb[:, ndc * 256 + eh * 128: ndc * 256 + (eh + 1) * 128],
                        Z[:, ndc * 16:(ndc + 1) * 16],
                        start=True, stop=True, skip_group_check=True)
            gt = wk.tile([128, 2 * BPC * N], BF16, tag="gt")
            for eh in range(2):
                dst = gt[:, eh * 256:(eh + 1) * 256].rearrange(
                    "p (b c two) -> p b c two", c=16, two=2)
                src = pgt[eh][:].rearrange("p (c b two) -> p b c two", b=8, two=2)
                rin = rbe[:].rearrange("p (c two b) -> p b c two", two=2, b=8)
                nc.vector.tensor_mul(dst, src, rin)
            return gt

        def emit_bnew_softmax_a(gt):
            """b=u@g^T -> softmax over n -> a-phase; returns at[p,(ec,b,n)] bf16."""
            pbt = [pp.tile([128, BPC * N], F32, tag="ps", name=f"pbt{_ic}")
                   for _ic in range(4)]
            for ic in range(4):
                for b in range(8):
                    for eh in range(2):
                        nc.tensor.matmul(
                            pbt[ic][:, b * N:(b + 1) * N],
                            uT_sb[:, eh * 4096 + b * I + ic * 128:
                                  eh * 4096 + b * I + (ic + 1) * 128],
                            gt[:, eh * 256 + b * N: eh * 256 + (b + 1) * N],
                            start=(eh == 0), stop=(eh == 1),
                            skip_group_check=True)
            esb = wk.tile([128, 4 * BPC * N], F32, tag="esb")
            for ic in range(4):
                nc.scalar.activation(esb[:, ic * 256:(ic + 1) * 256],
                                     pbt[ic][:], AF.Exp)
            ssum = wk.tile([128, 4 * BPC], F32, tag="ssum")
            nc.vector.reduce_sum(ssum[:],
                                 esb[:].rearrange("p (g n) -> p g n", n=N),
                                 axis=AX.X)
            rcp = wk.tile([128, 4 * BPC], F32, tag="rcp")
            nc.vector.reciprocal(rcp[:], ssum[:])
            csb = wk.tile([128, 4 * BPC * N], BF16, tag="csb")
            e3 = esb[:].rearrange("p (g n) -> p g n", n=N)
            r3 = rcp[:].rearrange("p (g o) -> p g o", o=1)
            e3b, r3b = bass.broadcast_tensor_aps(e3, r3)
            nc.vector.tensor_mul(csb[:].rearrange("p (g n) -> p g n", n=N),
                                 e3b, r3b)
            pat = pp.tile([128, 512], F32, tag="ps")
            for b in range(8):
                for eh in range(2):
                    for ic in range(4):
                        nc.tensor.matmul(
                            pat[:, b * 64 + eh * N: b * 64 + (eh + 1) * N],
                            u_sb[:, ic * 2048 + b * E + eh * 128:
                                 ic * 2048 + b * E + (eh + 1) * 128],
                            csb[:, ic * 256 + b * N: ic * 256 + (b + 1) * N],
                            start=(ic == 0), stop=(ic == 3),
                            skip_group_check=True)
            at = wk.tile([128, 2 * BPC * N], BF16, tag="at")
            nc.vector.tensor_copy(
                at[:].rearrange("p (eh b n) -> p b eh n", eh=2, n=N),
                pat[:].rearrange("p (b eh n) -> p b eh n", eh=2, n=N))
            return at

        def emit_final(o_flat):
            ps2 = emit_s2(o_flat)
            r_s = wk.tile([32, 8], F32, tag="r_s")    # sqrt(s2)
            nc.scalar.sqrt(r_s[:], ps2[:])
            onep = wk.tile([32, 8], F32, tag="onep")  # 1 + s2
            nc.scalar.add(onep[:], ps2[:], 1.0)
            rec = wk.tile([32, 8], F32, tag="rec")
            nc.vector.reciprocal(rec[:], onep[:])
            scl = wk.tile([32, 8], F32, tag="scl")    # sqrt(s2)/(1+s2)
            nc.vector.tensor_mul(scl[:], r_s[:], rec[:])
            sclf = wk.tile([1, 256], F32, tag="rsf")
            nc.sync.dma_start(sclf[:], scl[:])
            sbe = wk.tile([128, 256], F32, tag="rbe")
            nc.gpsimd.partition_broadcast(sbe[:], sclf[:])
            osc = wk.tile([128, 128], F32, tag="osc")
            for h in range(2):
                pv = slice(h * 64, (h + 1) * 64)
                sview = sbe[pv, :].rearrange("p (c g b) -> p c g b",
                                             g=2, b=8)[:, :, h, :]
                nc.vector.tensor_mul(
                    osc[pv, :].rearrange("p (c b) -> p c b", b=8),
                    o_flat[pv, :].rearrange("p (c b) -> p c b", b=8),
                    sview)
            ptr = pp.tile([128, 128], F32, tag="ps")
            nc.tensor.transpose(ptr[:], osc[:], ident_sb[:])
            trs = wk.tile([128, 128], F32, tag="trs")
            nc.vector.tensor_copy(trs[:], ptr[:])
            ov = out_d.ap().rearrange("b (c two) d -> two c b d", two=2)
            for h in range(2):
                nc.sync.dma_start(ov[h], trs[:, h * 64:(h + 1) * 64])

        o_flat = emit_o_phase(rhs_iter0)
        for _ in range(2):
            gt = emit_norm_g(o_flat)
            at = emit_bnew_softmax_a(gt)
            o_flat = emit_o_phase(
                lambda ec, at=at: at[:, ec * 256:(ec + 1) * 256])
        emit_final(o_flat)

    nc.compile()
    return nc


class _Runner:
    """Cached jitted SPMD executor (mirrors bass2jax.run_bass_via_pjrt)."""

    def __init__(self, nc):
        import jax
        import concourse.mybir as mybir
        from concourse import bass2jax
        from concourse.bass2jax import _bass_exec_p, install_neuronx_cc_hook
        from jax.sharding import Mesh, PartitionSpec
        from jax.experimental.shard_map import shard_map

        install_neuronx_cc_hook()
        self.jax = jax
        in_names, out_names, out_avals = [], [], []
        pname = nc.partition_id_tensor.name if nc.partition_id_tensor else None
        for alloc in nc.m.functions[0].allocations:
            if not isinstance(alloc, mybir.MemoryLocationSet):
                continue
            name = alloc.memorylocations[0].name
            if alloc.kind == "ExternalInput":
                if name != pname:
                    in_names.append(name)
            elif alloc.kind == "ExternalOutput":
                out_names.append(name)
                out_avals.append(jax.core.ShapedArray(
                    tuple(alloc.tensor_shape), mybir.dt.np(alloc.dtype)))
        self.in_names, self.out_names, self.out_avals = in_names, out_names, out_avals
        all_in = in_names + out_names + ([pname] if pname else [])
        n_params, n_outs = len(in_names), len(out_names)

        def _body(*args):
            operands = list(args)
            if pname is not None:
                operands.append(bass2jax.partition_id_tensor())
            return tuple(_bass_exec_p.bind(
                *operands, out_avals=tuple(out_avals), in_names=tuple(all_in),
                out_names=tuple(out_names), lowering_input_output_aliases=(),
                sim_require_finite=True, sim_require_nnan=True, nc=nc))

        devices = jax.devices()[:NCORES]
        mesh = Mesh(np.asarray(devices), ("core",))
        self._fn = jax.jit(
            shard_map(_body, mesh=mesh,
                      in_specs=(PartitionSpec("core"),) * (n_params + n_outs),
                      out_specs=(PartitionSpec("core"),) * n_outs,
                      check_rep=False),
            keep_unused=True)
        self._zeros = [np.zeros((NCORES * a.shape[0], *a.shape[1:]), a.dtype)
                       for a in out_avals]

    def run(self, per_core_inputs):
        concat = [np.concatenate([m[name] for m in per_core_inputs], axis=0)
                  for name in self.in_names]
        outs = self._fn(*concat, *self._zeros)
        self.jax.block_until_ready(outs)
        return [np.asarray(o) for o in outs]


@functools.lru_cache(maxsize=1)
def _get_runner():
    return _Runner(_build_module())


def _prep_inputs(u_vecs, W):
    u_vecs = np.ascontiguousarray(np.asarray(u_vecs, np.float32))
    W0 = np.ascontiguousarray(np.asarray(W, np.float32)[0])
    u4 = u_vecs.reshape(8, 8, 4, 128, 256).transpose(0, 3, 2, 1, 4) \
        .reshape(8, 128, 8192).astype(BF)
    uT4 = u_vecs.reshape(8, 8, 512, 2, 128).transpose(0, 4, 3, 1, 2) \
        .reshape(8, 128, 8192).astype(BF)
    w4 = W0.reshape(2, 128, 2048).transpose(1, 0, 2).reshape(128, 4096).astype(BF)
    wT4 = np.ascontiguousarray(W0.T).reshape(16, 128, 256) \
        .transpose(1, 0, 2).reshape(128, 4096).astype(BF)
    return [{"u_in": u4[c], "uT_in": uT4[c], "w_in": w4, "wT_in": wT4}
            for c in range(NCORES)]


def kernel(u_vecs: np.ndarray, W: np.ndarray) -> np.ndarray:
    runner = _get_runner()
    outs = runner.run(_prep_inputs(u_vecs, W))
    i = runner.out_names.index("out_d")
    return outs[i].reshape(B, N, D).astype(np.float32)
